# revision 1
# baseline (speedup 1.0000x reference)
"""Multi-head attention (L=2048, EMB=1024, H=16, D=64) on 8 TRN2 NeuronCores.

Tensor-parallel over heads: core i owns heads {2i, 2i+1} (a 128-row block of
Wq/Wk/Wv and a 128-column block of Wo). Each core computes its two heads'
attention plus its partial output projection; the host sums the 8 partials.

Device-side layout is fully transposed (scores^T = [m, l]) so no on-device
transposes are needed:
  QT[d, l] = (Wq_shard @ q^T)        lhsT = (Wq_shard/8)^T, rhs = q^T
  KT[d, l] = (Wk_shard @ k^T)
  V [m, d] = (v @ Wv_shard^T)        lhsT = v^T tile,       rhs = Wv_shard^T
  sT[m, l] = KT_h^T @ QT_h           (per head, contraction d=64)
  pT       = exp(sT) * keepT         (no max-subtraction: |s| <~ 9)
  attnT/Z  = [V_h | 1]^T @ pT        (ones column gives softmax denominator)
  outT     = Wo_shard^T-block @ (attnT / Z)   partial, summed on host

All matmuls run in bf16 (fp32 PSUM accumulation); measured end-to-end
relative error vs the fp32 reference is ~0.6%.

Pipeline structure (all tuned against neuron-profile NTFF traces):
- l-tile-major attention; each l-tile's epilogue (softmax-denominator
  normalize + output projection + store) is deferred and drip-fed as
  filler into the NEXT l-tile's quad stream, so the multi-hop Z DMA
  chain never blocks the in-order engine queues.
- exp() is batched over 3 key-tiles per ACTIVATE ((N+352)/1.2ns cost).
- One-stage software pipeline on the PE queue: quad q's attn matmuls
  are emitted after quad q+1's scores, decoupling PE from the
  exp->mask-mult chain.
- K/V/Q projection rounds are interleaved into the first head's
  attention stream; input DMAs are split across the sync/scalar/gpsimd
  rings in consumption order; mask chunks prefetch one head ahead.
- The softmax reciprocal is spread across 8 partitions via a DRAM
  bounce (single-partition DVE reciprocal costs ~6ns/element).
"""

import sys

for _p in ("/opt/trn_rl_repo",):
    if _p not in sys.path:
        sys.path.insert(0, _p)

from contextlib import ExitStack

import ml_dtypes
import numpy as np

import concourse.bass as bass
import concourse.tile as tile
from concourse import bacc, mybir
from concourse._compat import with_exitstack
from concourse.bass_utils import run_bass_kernel_spmd

BF16 = mybir.dt.bfloat16
FP8 = mybir.dt.float8e4
F32 = mybir.dt.float32
NPBF16 = ml_dtypes.bfloat16

L = 2048
EMB = 1024
NHEAD = 16
HEAD_DIM = 64
NCORES = 8
HPC = NHEAD // NCORES  # heads per core = 2
ROWS = HPC * HEAD_DIM  # weight rows per core = 128
SCALE = HEAD_DIM ** -0.5

LT = 512               # l-tile (matmul free dim / PSUM bank)
NLT = L // LT          # 4
MT = 128               # m-tile (key-block on partitions)
NMT = L // MT          # 16
ET = 128               # contraction tile over EMB
NET = EMB // ET        # 8
JT = 128               # output-row tile
NJT = EMB // JT        # 8

QUADS = (3, 3, 3, 3, 2, 2)   # m-tiles per exp/mask-mult instruction
QB = 3                        # psc tile m-capacity (PSUM banks per slot)
PSC_BUFS = 2


@with_exitstack
def _mha_kernel(ctx, tc, outT, qT, kT, vT, wqT, wkT, wvT, woT, maskT, dbg=None):
    nc = tc.nc

    const = ctx.enter_context(tc.tile_pool(name="const", bufs=1))
    ppool = ctx.enter_context(tc.tile_pool(name="ptiles", bufs=5))
    maskp = ctx.enter_context(tc.tile_pool(name="maskp", bufs=2))
    stage = ctx.enter_context(tc.tile_pool(name="stage", bufs=4))
    zpool = ctx.enter_context(tc.tile_pool(name="zpool", bufs=2))
    psc = ctx.enter_context(tc.tile_pool(name="psc", bufs=PSC_BUFS, space="PSUM"))
    psa = ctx.enter_context(tc.tile_pool(name="psa", bufs=2, space="PSUM"))

    # ---- resident input tiles; DMAs emitted in consumption order ----
    qTs = const.tile([128, NET, L], BF16, tag="qTs")
    kTs = const.tile([128, NET, L], BF16, tag="kTs")
    vTs = const.tile([128, NET, L], BF16, tag="vTs")
    wqs = const.tile([128, NET, ROWS], BF16, tag="wqs")
    wks = const.tile([128, NET, ROWS], BF16, tag="wks")
    wvs = const.tile([128, NET, ROWS], BF16, tag="wvs")
    wos = const.tile([128, EMB], BF16, tag="wos")  # [hd, j]
    q3 = qT.rearrange("(o p) l -> p o l", p=128)
    k3 = kT.rearrange("(o p) l -> p o l", p=128)
    v3 = vT.rearrange("(o p) l -> p o l", p=128)

    def chunk_dma(dst, src3, lc):
        nc.sync.dma_start(dst[:, :, bass.ts(lc, LT)], src3[:, :, bass.ts(lc, LT)])

    state = {}
    mask3 = maskT.rearrange("h (mo p) l -> h p mo l", p=128)

    def mask_fetch(lt, h, eng=None):
        mc = maskp.tile([128, NMT, LT], BF16, tag="maskc", name=f"maskc_{lt}_{h}")
        (eng or nc.scalar).dma_start(mc[:], mask3[h, :, :, bass.ts(lt, LT)])
        state[lt, h, "maskc"] = mc

    def chunk_dma_eng(eng, dst, src3, lc):
        eng.dma_start(dst[:, :, bass.ts(lc, LT)], src3[:, :, bass.ts(lc, LT)])

    # critical first chunks on the scalar HWDGE ring (short queue)
    nc.scalar.dma_start(wqs[:], wqT[:])
    chunk_dma_eng(nc.scalar, qTs, q3, 0)
    nc.scalar.dma_start(wks[:], wkT[:])
    chunk_dma_eng(nc.scalar, kTs, k3, 0)
    nc.scalar.dma_start(wvs[:], wvT[:])
    chunk_dma_eng(nc.scalar, vTs, v3, 0)
    # bulk tail chunks on the gpsimd SWDGE ring
    for lc in (2, 3):
        chunk_dma_eng(nc.gpsimd, kTs, k3, lc)
        chunk_dma_eng(nc.gpsimd, vTs, v3, lc)
    mask_fetch(0, 0, eng=nc.sync)
    # the rest on sync, behind the first mask chunk
    chunk_dma_eng(nc.sync, kTs, k3, 1)
    chunk_dma_eng(nc.sync, vTs, v3, 1)
    for lc in range(1, NLT):
        chunk_dma_eng(nc.sync, qTs, q3, lc)
    nc.sync.dma_start(wos[:], woT[:])

    QTb = const.tile([128, L], BF16, tag="QTb")
    KTb = const.tile([128, L], BF16, tag="KTb")
    VROW = 66
    vaug = const.tile([128, HPC, NMT, VROW], BF16, tag="vaug")
    nc.vector.memset(vaug[:, :, :, HEAD_DIM : HEAD_DIM + 1], 1.0)
    nc.vector.memset(vaug[:, :, :, HEAD_DIM + 1 : VROW], 0.0)

    def qk_proj(dst, w, x, lt):
        ps = psc.tile([128, QB, LT], F32, tag="psc", name="ps_proj")[:, 0, :]
        for et in range(NET):
            nc.tensor.matmul(
                ps[:],
                lhsT=w[:, et, :],
                rhs=x[:, et, bass.ts(lt, LT)],
                start=(et == 0),
                stop=(et == NET - 1),
            )
        nc.vector.tensor_copy(out=dst[:, bass.ts(lt, LT)], in_=ps[:])

    def v_proj(mt):
        ps = psc.tile([128, QB, LT], F32, tag="psc", name="ps_v")[:, 0, :ROWS]
        for et in range(NET):
            nc.tensor.matmul(
                ps[:],
                lhsT=vTs[:, et, bass.ts(mt, MT)],
                rhs=wvs[:, et, :],
                start=(et == 0),
                stop=(et == NET - 1),
            )
        for h in range(HPC):
            nc.vector.tensor_copy(
                out=vaug[:, h, mt, 0:HEAD_DIM],
                in_=ps[:, bass.ts(h, HEAD_DIM)],
            )

    # ---- attention + per-l-tile epilogue ----
    attnTb = const.tile([128, L], BF16, tag="attnTb")

    zdram = nc.dram_tensor("zdram", [NLT, HPC * LT], F32).ap()
    zidram = nc.dram_tensor("zidram", [NLT, HPC * LT], F32).ap()

    # Epilogue work for l-tile X is deferred and drip-fed as PE/DVE filler
    # into l-tile X+1's quad stream, so the z-chain DMA latency never sits
    # in front of the in-order engine queues.
    pending = []

    def piece_zload(lt):
        def go():
            zsp = zpool.tile([8, HPC * LT // 8], F32, tag="zsp", name=f"zsp_{lt}")
            nc.sync.dma_start(zsp[:], zdram[lt].rearrange("(o p) -> o p", o=8))
            state[lt, "zsp"] = zsp
        return go

    def piece_recip_bcast(lt):
        def go():
            zsp = state[lt, "zsp"]
            nc.vector.reciprocal(zsp[:], zsp[:])
            nc.sync.dma_start(zidram[lt].rearrange("(o p) -> o p", o=8), zsp[:])
            zinvb = zpool.tile([128, LT], F32, tag="zinvb", name=f"zinvb_{lt}")
            for h in range(HPC):
                nc.sync.dma_start(
                    zinvb[bass.ts(h, HEAD_DIM), :],
                    zidram[lt][None, bass.ts(h, LT)].to_broadcast((HEAD_DIM, LT)),
                )
            state[lt, "zinvb"] = zinvb
        return go

    def piece_norm(lt):
        def go():
            ls = bass.ts(lt, LT)
            nc.vector.tensor_mul(
                out=attnTb[:, ls], in0=attnTb[:, ls], in1=state[lt, "zinvb"][:]
            )
        return go

    def piece_outproj(lt, jt):
        def go():
            ls = bass.ts(lt, LT)
            ps = psc.tile([128, QB, LT], F32, tag="psc", name="ps_out")[:, 0, :]
            nc.tensor.matmul(
                ps[:],
                lhsT=wos[:, bass.ts(jt, JT)],
                rhs=attnTb[:, ls],
                start=True,
                stop=True,
            )
            st = stage.tile([128, LT], F32, tag="st", name="st")
            nc.vector.tensor_copy(out=st[:], in_=ps[:])
            nc.gpsimd.dma_start(outT[bass.ts(jt, JT), ls], st[:])
        return go

    qk_proj(QTb, wqs, qTs, 0)

    def mask_fetch(lt, h, eng=None):
        mc = maskp.tile([128, NMT, LT], BF16, tag="maskc", name=f"maskc_{lt}_{h}")
        (eng or nc.scalar).dma_start(mc[:], mask3[h, :, :, bass.ts(lt, LT)])
        state[lt, h, "maskc"] = mc


    for lt in range(NLT):
        ls = bass.ts(lt, LT)
        zseg = zpool.tile([128, HPC * LT], F32, tag="zseg", name=f"zseg_{lt}")
        for h in range(HPC):
            hd = bass.ts(h, HEAD_DIM)
            nxt = lt * HPC + h + 1
            if nxt < NLT * HPC:
                mask_fetch(nxt // HPC, nxt % HPC)
            maskc = state[lt, h, "maskc"]
            pa = psa.tile([128, LT], F32, tag="psa", name=f"psa_{lt}_{h}")
            mt0 = 0
            prev_attn = None
            for qi, qn in enumerate(QUADS):
                if lt == 0 and h == 0:
                    # interleave K/V projection rounds into the first
                    # attention stream so the PE queue never drains
                    for mt in range(mt0, mt0 + qn):
                        if mt % (LT // MT) == 0:
                            qk_proj(KTb, wks, kTs, mt // (LT // MT))
                        v_proj(mt)
                if lt == 0 and h == 1 and 1 <= qi <= 3:
                    qk_proj(QTb, wqs, qTs, qi)  # PE filler + needed later
                if pending:
                    pending[0][0] -= 1
                    if pending[0][0] < 0:
                        pending.pop(0)[1]()
                ss = psc.tile([128, QB, LT], F32, tag="psc", name="ss")
                for i in range(qn):
                    nc.tensor.matmul(
                        ss[:, i, :],
                        lhsT=KTb[hd, bass.ts(mt0 + i, MT)],
                        rhs=QTb[hd, ls],
                        start=True,
                        stop=True,
                    )
                # one-stage software pipeline on PE: the previous quad's
                # attn matmuls are emitted AFTER this quad's scores, so the
                # in-order PE queue never blocks scores behind the
                # exp->mask-mult chain of the previous quad
                if prev_attn is not None:
                    prev_attn()
                pT = ppool.tile([128, QB, LT], BF16, tag="pT", name="pT")
                nc.scalar.activation(
                    pT[:, :qn, :], ss[:, :qn, :], mybir.ActivationFunctionType.Exp
                )
                nc.vector.tensor_mul(
                    out=pT[:, :qn, :], in0=pT[:, :qn, :],
                    in1=maskc[:, mt0 : mt0 + qn, :],
                )

                def make_attn(mt0=mt0, qn=qn, pT=pT):
                    def go():
                        for i in range(qn):
                            mt = mt0 + i
                            nc.tensor.matmul(
                                pa[:VROW, :],
                                lhsT=vaug[:, h, mt, :],
                                rhs=pT[:, i, :],
                                start=(mt == 0),
                                stop=(mt == NMT - 1),
                            )
                    return go

                prev_attn = make_attn()
                mt0 += qn
            prev_attn()
            nc.vector.tensor_copy(out=attnTb[hd, ls], in_=pa[0:HEAD_DIM, :])
            nc.vector.tensor_copy(
                out=zseg[HEAD_DIM : HEAD_DIM + 1, bass.ts(h, LT)],
                in_=pa[HEAD_DIM : HEAD_DIM + 1, :],
            )
        nc.sync.dma_start(zdram[lt][None, :], zseg[HEAD_DIM : HEAD_DIM + 1, :])
        pending.append([1, piece_zload(lt)])
        pending.append([0, piece_recip_bcast(lt)])
        pending.append([2, piece_norm(lt)])
        for jt in range(NJT):
            pending.append([0, piece_outproj(lt, jt)])

    while pending:
        pending.pop(0)[1]()

    if dbg is not None:
        nc.sync.dma_start(dbg["QTb"][:], QTb[:])
        nc.sync.dma_start(dbg["KTb"][:], KTb[:])
        nc.sync.dma_start(dbg["vaug"][:], vaug[:])
        nc.sync.dma_start(dbg["attnTb_post"][:], attnTb[:])


_CACHE = {}


def _build(debug=False):
    key = ("nc", debug)
    if key in _CACHE:
        return _CACHE[key]
    nc = bacc.Bacc("TRN2", target_bir_lowering=False, debug=False,
                   num_devices=NCORES)
    qT = nc.dram_tensor("qT", [EMB, L], BF16, kind="ExternalInput").ap()
    kT = nc.dram_tensor("kT", [EMB, L], BF16, kind="ExternalInput").ap()
    vT = nc.dram_tensor("vT", [EMB, L], BF16, kind="ExternalInput").ap()
    wqT = nc.dram_tensor("wqT", [128, NET, ROWS], BF16, kind="ExternalInput").ap()
    wkT = nc.dram_tensor("wkT", [128, NET, ROWS], BF16, kind="ExternalInput").ap()
    wvT = nc.dram_tensor("wvT", [128, NET, ROWS], BF16, kind="ExternalInput").ap()
    woT = nc.dram_tensor("woT", [ROWS, EMB], BF16, kind="ExternalInput").ap()
    maskT = nc.dram_tensor("maskT", [HPC, L, L], BF16, kind="ExternalInput").ap()
    outT = nc.dram_tensor("outT", [EMB, L], F32, kind="ExternalOutput").ap()
    dbg = None
    if debug:
        dbg = {
            "QTb": nc.dram_tensor("dbg_QTb", [128, L], BF16, kind="ExternalOutput").ap(),
            "KTb": nc.dram_tensor("dbg_KTb", [128, L], BF16, kind="ExternalOutput").ap(),
            "vaug": nc.dram_tensor("dbg_vaug", [128, NMT, HPC, HEAD_DIM + 1], BF16, kind="ExternalOutput").ap(),
            "attnTb_pre": nc.dram_tensor("dbg_attnTb_pre", [128, L], BF16, kind="ExternalOutput").ap(),
            "attnTb_post": nc.dram_tensor("dbg_attnTb_post", [128, L], BF16, kind="ExternalOutput").ap(),
            "zinvb": nc.dram_tensor("dbg_zinvb", [128, L], F32, kind="ExternalOutput").ap(),
        }

    with tile.TileContext(nc) as tc:
        _mha_kernel(tc, outT, qT, kT, vT, wqT, wkT, wvT, woT, maskT, dbg=dbg)
    nc.compile()
    _CACHE[key] = nc
    return nc


def _pack_w(w):
    # [ROWS, EMB] -> w.T [EMB, ROWS] -> [128, NET, ROWS] with e = o*128+p
    return np.ascontiguousarray(
        w.T.reshape(NET, 128, ROWS).transpose(1, 0, 2)
    ).astype(NPBF16)


def _prep_in_maps(q, k, v, mask, Wq, Wk, Wv, Wo):
    qT = np.ascontiguousarray(q.T).astype(NPBF16)
    kT = np.ascontiguousarray(k.T).astype(NPBF16)
    vT = np.ascontiguousarray(v.T).astype(NPBF16)
    in_maps = []
    for c in range(NCORES):
        rows = slice(c * ROWS, (c + 1) * ROWS)
        in_maps.append({
            "qT": qT,
            "kT": kT,
            "vT": vT,
            "wqT": _pack_w(Wq[rows] * SCALE),
            "wkT": _pack_w(Wk[rows]),
            "wvT": _pack_w(Wv[rows]),
            "woT": np.ascontiguousarray(Wo[:, rows].T).astype(NPBF16),
            "maskT": np.ascontiguousarray(
                (~mask[c * HPC : (c + 1) * HPC]).swapaxes(1, 2)
            ).astype(NPBF16),
        })
    return in_maps


def run(q, k, v, mask, Wq, Wk, Wv, Wo, debug=False, **spmd_kwargs):
    nc = _build(debug=debug)
    in_maps = _prep_in_maps(q, k, v, mask, Wq, Wk, Wv, Wo)
    res = run_bass_kernel_spmd(nc, in_maps, list(range(NCORES)), **spmd_kwargs)
    outT = np.zeros((EMB, L), np.float64)
    for r in res.results:
        outT += r["outT"].astype(np.float64)
    out = np.ascontiguousarray(outT.T).astype(np.float32)
    return out, res


def kernel(q, k, v, mask, Wq, Wk, Wv, Wo):
    q, k, v = (np.asarray(x, np.float32) for x in (q, k, v))
    Wq, Wk, Wv, Wo = (np.asarray(x, np.float32) for x in (Wq, Wk, Wv, Wo))
    mask = np.asarray(mask, bool)
    out, _ = run(q, k, v, mask, Wq, Wk, Wv, Wo)
    return out



# revision 6
# speedup vs baseline: 1.0326x; 1.0326x over previous
"""Multi-head attention (L=2048, EMB=1024, H=16, D=64) on 8 TRN2 NeuronCores.

Tensor-parallel over heads: core i owns heads {2i, 2i+1} (a 128-row block of
Wq/Wk/Wv and a 128-column block of Wo). Each core computes its two heads'
attention plus its partial output projection; the host sums the 8 partials.

Device-side layout is fully transposed (scores^T = [m, l]) so no on-device
transposes are needed:
  QT[d, l] = (Wq_shard @ q^T)        lhsT = (Wq_shard/8)^T, rhs = q^T
  KT[d, l] = (Wk_shard @ k^T)
  V [m, d] = (v @ Wv_shard^T)        lhsT = v^T tile,       rhs = Wv_shard^T
  sT[m, l] = KT_h^T @ QT_h           (per head, contraction d=64)
  pT       = exp(sT) * keepT         (no max-subtraction: |s| <~ 9)
  attnT|Z  = [V_h | 1*64]^T @ pT     (ones cols 64:128 broadcast the softmax
                                      denominator Z onto PSUM rows 64:127)
  attnT/Z  = pa[0:64] * recip(pa[64:128])   local DVE, no DRAM bounce
  outT     = Wo_shard^T-block @ (attnT / Z)   bf16 partial, summed on host

All matmuls run in bf16 (fp32 PSUM accumulation); measured end-to-end
relative error vs the fp32 reference is ~0.6%.

Schedule notes (tuned against neuron-profile NTFF traces + the CoreSim
cost model):
- PE clock ramps 0.65 -> 1.2 -> 2.4 GHz with sustained use; every idle gap
  resets the ramp, so the whole schedule aims to keep the PE queue fed.
- The mask ships as fp8e4 (8 MB/core) and is upcast to bf16 in-flight by
  the gpsimd SWDGE DMA, so the DVE multiply keeps its 2x 16-bit rate.
- Output partials are stored bf16 (host sums in f64): halves store traffic.
- Critical-path input DMAs are split across rings: scalar gets {wq, q},
  sync gets {wk, k, wv, v, wo}, gpsimd gets mask fetches (half-tiles, one
  pass ahead), so the first scores matmul issues ~12us in.
- One-stage software pipeline on the PE queue: quad q's attn matmuls are
  emitted after quad q+1's scores, decoupling PE from the exp->mask-mult
  chain; exp is batched 3 key-tiles per ACTIVATE.
- Per-l-tile output projection is drip-fed one piece per quad into the
  next passes' streams as PE filler.
"""

import sys

for _p in ("/opt/trn_rl_repo",):
    if _p not in sys.path:
        sys.path.insert(0, _p)

from contextlib import ExitStack

import ml_dtypes
import numpy as np

import concourse.bass as bass
import concourse.tile as tile
from concourse import bacc, mybir
from concourse._compat import with_exitstack
from concourse.bass_utils import run_bass_kernel_spmd

BF16 = mybir.dt.bfloat16
FP8 = mybir.dt.float8e4
F32 = mybir.dt.float32
NPBF16 = ml_dtypes.bfloat16
NPFP8 = ml_dtypes.float8_e4m3

L = 2048
EMB = 1024
NHEAD = 16
HEAD_DIM = 64
NCORES = 8
HPC = NHEAD // NCORES  # heads per core = 2
ROWS = HPC * HEAD_DIM  # weight rows per core = 128
SCALE = HEAD_DIM ** -0.5

LT = 512               # l-tile (matmul free dim / PSUM bank)
NLT = L // LT          # 4
MT = 128               # m-tile (key-block on partitions)
NMT = L // MT          # 16
ET = 128               # contraction tile over EMB
NET = EMB // ET        # 8
JT = 128               # output-row tile
NJT = EMB // JT        # 8

# (mt0, qn) per quad; halves split at mt=8 to match mask half-tiles
QUADS = ((0, 3), (3, 3), (6, 2), (8, 3), (11, 3), (14, 2))
QB = 3                 # psc tile m-capacity (PSUM banks per slot)
VROW = 128             # attn lhsT free dim: 64 V rows + 64 ones rows (Z bcast)


@with_exitstack
def _mha_kernel(ctx, tc, outT, qT, kT, vT, wqT, wkT, wvT, woT, maskT):
    nc = tc.nc

    const = ctx.enter_context(tc.tile_pool(name="const", bufs=1))
    ppool = ctx.enter_context(tc.tile_pool(name="ptiles", bufs=5))
    maskp = ctx.enter_context(tc.tile_pool(name="maskp", bufs=4))
    stage = ctx.enter_context(tc.tile_pool(name="stage", bufs=4))
    zpool = ctx.enter_context(tc.tile_pool(name="zpool", bufs=2))
    psc = ctx.enter_context(tc.tile_pool(name="psc", bufs=2, space="PSUM"))
    psa = ctx.enter_context(tc.tile_pool(name="psa", bufs=1, space="PSUM"))
    pso = ctx.enter_context(tc.tile_pool(name="pso", bufs=1, space="PSUM"))

    # ---- resident input tiles; DMAs split across rings in consumption order
    qTs = const.tile([128, NET, L], BF16, tag="qTs")
    kTs = const.tile([128, NET, L], BF16, tag="kTs")
    vTs = const.tile([128, NET, L], BF16, tag="vTs")
    wqs = const.tile([128, NET, ROWS], BF16, tag="wqs")
    wks = const.tile([128, NET, ROWS], BF16, tag="wks")
    wvs = const.tile([128, NET, ROWS], BF16, tag="wvs")
    wos = const.tile([128, EMB], BF16, tag="wos")  # [hd, j]
    q3 = qT.rearrange("(o p) l -> p o l", p=128)
    k3 = kT.rearrange("(o p) l -> p o l", p=128)
    v3 = vT.rearrange("(o p) l -> p o l", p=128)
    mask3 = maskT.rearrange("h (mo p) l -> h p mo l", p=128)
    out3 = outT.rearrange("(b p) l -> p b l", p=128)

    def chunk(eng, dst, src3, lc):
        eng.dma_start(dst[:, :, bass.ts(lc, LT)], src3[:, :, bass.ts(lc, LT)])

    # scalar ring: q-side critical path, then q tail; nothing else ever
    # (keeps the ACT engine free for exp)
    nc.scalar.dma_start(wqs[:], wqT[:])
    chunk(nc.scalar, qTs, q3, 0)
    for lc in range(1, NLT):
        chunk(nc.scalar, qTs, q3, lc)
    # sync ring: k/v-side critical path, then bulk
    nc.sync.dma_start(wks[:], wkT[:])
    chunk(nc.sync, kTs, k3, 0)
    nc.sync.dma_start(wvs[:], wvT[:])
    chunk(nc.sync, vTs, v3, 0)
    for lc in range(1, NLT):
        chunk(nc.sync, kTs, k3, lc)
        chunk(nc.sync, vTs, v3, lc)
    nc.sync.dma_start(wos[:], woT[:])

    state = {}

    def mask_fetch(lt, h, half):
        mc = maskp.tile([128, 8, LT], BF16, tag="maskc",
                        name=f"maskc_{lt}_{h}_{half}")
        nc.gpsimd.dma_start(
            mc[:], mask3[h, :, 8 * half : 8 * half + 8, bass.ts(lt, LT)]
        )
        state[lt, h, half] = mc

    mask_fetch(0, 0, 0)
    mask_fetch(0, 0, 1)

    QTb = const.tile([128, L], BF16, tag="QTb")
    KTb = const.tile([128, L], BF16, tag="KTb")
    vaug = const.tile([128, HPC, NMT, VROW], BF16, tag="vaug")
    nc.vector.memset(vaug[:, :, :, HEAD_DIM:VROW], 1.0)
    attnTb = const.tile([128, L], BF16, tag="attnTb")

    def qk_proj(dst, w, x, lt, ps, use_act):
        for et in range(NET):
            nc.tensor.matmul(
                ps[:],
                lhsT=w[:, et, :],
                rhs=x[:, et, bass.ts(lt, LT)],
                start=(et == 0),
                stop=(et == NET - 1),
            )
        if use_act:
            nc.scalar.copy(out=dst[:, bass.ts(lt, LT)], in_=ps[:])
        else:
            nc.vector.tensor_copy(out=dst[:, bass.ts(lt, LT)], in_=ps[:])

    def v_proj(mt):
        # NOTE: PSUM accumulation groups are bank-granular; only one open
        # group per bank at a time (interleaving two corrupts both).
        ps = pso.tile([128, LT], F32, tag="pso", name="ps_v")
        for et in range(NET):
            nc.tensor.matmul(
                ps[:, :ROWS],
                lhsT=vTs[:, et, bass.ts(mt, MT)],
                rhs=wvs[:, et, :],
                start=(et == 0),
                stop=(et == NET - 1),
            )
        for h in range(HPC):
            nc.vector.tensor_copy(
                out=vaug[:, h, mt, 0:HEAD_DIM],
                in_=ps[:, bass.ts(h, HEAD_DIM)],
            )

    def k_proj(lc):
        ps = pso.tile([128, LT], F32, tag="pso", name="ps_k")
        qk_proj(KTb, wks, kTs, lc, ps, use_act=True)

    def q_proj(lc):
        ps = pso.tile([128, LT], F32, tag="pso", name="ps_q")
        qk_proj(QTb, wqs, qTs, lc, ps, use_act=False)

    # ---- prologue: Q(lt0) and K(chunk0) projections on separate psc slots
    ps_q0 = psc.tile([128, QB, LT], F32, tag="psc", name="ps_q0")[:, 0, :]
    qk_proj(QTb, wqs, qTs, 0, ps_q0, use_act=True)
    ps_k0 = psc.tile([128, QB, LT], F32, tag="psc", name="ps_k0")[:, 0, :]
    qk_proj(KTb, wks, kTs, 0, ps_k0, use_act=True)

    # pass-0 PE filler: (quad index) -> emit projections whose inputs have
    # landed by then; K chunk b feeds scores of quads covering mt>=4b,
    # v pair (m,m+1) feeds the attn matmul emitted one quad later.
    pass00_filler = {
        0: [lambda: v_proj(0), lambda: v_proj(1)],
        1: [lambda: k_proj(1), lambda: v_proj(2), lambda: v_proj(3)],
        2: [lambda: v_proj(4), lambda: v_proj(5)],
        3: [lambda: k_proj(2), lambda: v_proj(6), lambda: v_proj(7)],
        4: [lambda: k_proj(3)] + [lambda mt=mt: v_proj(mt) for mt in (8, 9, 10, 11)],
        5: [lambda mt=mt: v_proj(mt) for mt in (12, 13, 14, 15)],
    }
    pass01_filler = {
        1: [lambda: q_proj(1)],
        2: [lambda: q_proj(2)],
        3: [lambda: q_proj(3)],
    }

    # deferred per-l-tile output projection, drip-fed as PE filler
    pending = []

    def piece_outproj(lt, jt, do_store):
        def go():
            ls = bass.ts(lt, LT)
            if (lt, "st") not in state:
                state[lt, "st"] = stage.tile([128, 2, LT], BF16, tag="st",
                                             name=f"st_{lt}_{jt}")
            st = state[lt, "st"]
            ps = pso.tile([128, LT], F32, tag="pso", name="ps_out")
            nc.tensor.matmul(
                ps[:],
                lhsT=wos[:, bass.ts(jt, JT)],
                rhs=attnTb[:, ls],
                start=True,
                stop=True,
            )
            nc.vector.tensor_copy(out=st[:, jt % 2, :], in_=ps[:])
            if do_store:
                nc.sync.dma_start(out3[:, jt - 1 : jt + 1, ls], st[:])
                del state[lt, "st"]
        return go

    for lt in range(NLT):
        ls = bass.ts(lt, LT)
        for h in range(HPC):
            hd = bass.ts(h, HEAD_DIM)
            pa = psa.tile([128, LT], F32, tag="psa", name=f"psa_{lt}_{h}")
            prev_attn = None
            for qi, (mt0, qn) in enumerate(QUADS):
                if lt == 0 and h == 0:
                    for f in pass00_filler.get(qi, ()):
                        f()
                if lt == 0 and h == 1:
                    for f in pass01_filler.get(qi, ()):
                        f()
                # prefetch next pass's mask halves: A at quad 0, B at quad 3
                nxt = lt * HPC + h + 1
                if nxt < NLT * HPC and qi in (0, 3):
                    mask_fetch(nxt // HPC, nxt % HPC, 0 if qi == 0 else 1)
                if pending:
                    pending.pop(0)()
                ss = psc.tile([128, QB, LT], F32, tag="psc", name="ss")
                for i in range(qn):
                    nc.tensor.matmul(
                        ss[:, i, :],
                        lhsT=KTb[hd, bass.ts(mt0 + i, MT)],
                        rhs=QTb[hd, ls],
                        start=True,
                        stop=True,
                    )
                # one-stage software pipeline on PE: the previous quad's
                # attn matmuls are emitted AFTER this quad's scores
                if prev_attn is not None:
                    prev_attn()
                pT = ppool.tile([128, QB, LT], BF16, tag="pT", name="pT")
                nc.scalar.activation(
                    pT[:, :qn, :], ss[:, :qn, :], mybir.ActivationFunctionType.Exp
                )
                maskc = state[lt, h, 0 if mt0 < 8 else 1]
                j0 = mt0 if mt0 < 8 else mt0 - 8
                nc.vector.tensor_mul(
                    out=pT[:, :qn, :], in0=pT[:, :qn, :],
                    in1=maskc[:, j0 : j0 + qn, :],
                )

                def make_attn(mt0=mt0, qn=qn, pT=pT):
                    def go():
                        for i in range(qn):
                            mt = mt0 + i
                            nc.tensor.matmul(
                                pa[:],
                                lhsT=vaug[:, h, mt, :],
                                rhs=pT[:, i, :],
                                start=(mt == 0),
                                stop=(mt == NMT - 1),
                            )
                    return go

                prev_attn = make_attn()
            prev_attn()
            # local softmax normalize: rows 64:127 of pa all hold Z
            zinv = zpool.tile([64, LT], F32, tag="zinv", name=f"zinv_{lt}_{h}")
            nc.vector.reciprocal(zinv[:], pa[64:128, :])
            nc.vector.tensor_mul(
                out=attnTb[hd, ls], in0=pa[0:HEAD_DIM, :], in1=zinv[:]
            )
        for jt in range(NJT):
            pending.append(piece_outproj(lt, jt, do_store=(jt % 2 == 1)))

    while pending:
        pending.pop(0)()


_CACHE = {}


def _build():
    if "nc" in _CACHE:
        return _CACHE["nc"]
    nc = bacc.Bacc("TRN2", target_bir_lowering=False, debug=False,
                   num_devices=NCORES)
    qT = nc.dram_tensor("qT", [EMB, L], BF16, kind="ExternalInput").ap()
    kT = nc.dram_tensor("kT", [EMB, L], BF16, kind="ExternalInput").ap()
    vT = nc.dram_tensor("vT", [EMB, L], BF16, kind="ExternalInput").ap()
    wqT = nc.dram_tensor("wqT", [128, NET, ROWS], BF16, kind="ExternalInput").ap()
    wkT = nc.dram_tensor("wkT", [128, NET, ROWS], BF16, kind="ExternalInput").ap()
    wvT = nc.dram_tensor("wvT", [128, NET, ROWS], BF16, kind="ExternalInput").ap()
    woT = nc.dram_tensor("woT", [ROWS, EMB], BF16, kind="ExternalInput").ap()
    maskT = nc.dram_tensor("maskT", [HPC, L, L], FP8, kind="ExternalInput").ap()
    outT = nc.dram_tensor("outT", [EMB, L], BF16, kind="ExternalOutput").ap()

    with tile.TileContext(nc) as tc:
        _mha_kernel(tc, outT, qT, kT, vT, wqT, wkT, wvT, woT, maskT)
    nc.compile()
    _CACHE["nc"] = nc
    return nc


def _pack_w(w):
    # [ROWS, EMB] -> w.T [EMB, ROWS] -> [128, NET, ROWS] with e = o*128+p
    return np.ascontiguousarray(
        w.T.reshape(NET, 128, ROWS).transpose(1, 0, 2)
    ).astype(NPBF16)


def _prep_in_maps(q, k, v, mask, Wq, Wk, Wv, Wo):
    qT = np.ascontiguousarray(q.T).astype(NPBF16)
    kT = np.ascontiguousarray(k.T).astype(NPBF16)
    vT = np.ascontiguousarray(v.T).astype(NPBF16)
    in_maps = []
    for c in range(NCORES):
        rows = slice(c * ROWS, (c + 1) * ROWS)
        in_maps.append({
            "qT": qT,
            "kT": kT,
            "vT": vT,
            "wqT": _pack_w(Wq[rows] * SCALE),
            "wkT": _pack_w(Wk[rows]),
            "wvT": _pack_w(Wv[rows]),
            "woT": np.ascontiguousarray(Wo[:, rows].T).astype(NPBF16),
            "maskT": np.ascontiguousarray(
                (~mask[c * HPC : (c + 1) * HPC]).swapaxes(1, 2)
            ).astype(NPFP8),
        })
    return in_maps


def run(q, k, v, mask, Wq, Wk, Wv, Wo, **spmd_kwargs):
    nc = _build()
    in_maps = _prep_in_maps(q, k, v, mask, Wq, Wk, Wv, Wo)
    res = run_bass_kernel_spmd(nc, in_maps, list(range(NCORES)), **spmd_kwargs)
    outT = np.zeros((EMB, L), np.float64)
    for r in res.results:
        outT += np.asarray(r["outT"]).astype(np.float64)
    out = np.ascontiguousarray(outT.T).astype(np.float32)
    return out, res


def kernel(q, k, v, mask, Wq, Wk, Wv, Wo):
    q, k, v = (np.asarray(x, np.float32) for x in (q, k, v))
    Wq, Wk, Wv, Wo = (np.asarray(x, np.float32) for x in (Wq, Wk, Wv, Wo))
    mask = np.asarray(mask, bool)
    out, _ = run(q, k, v, mask, Wq, Wk, Wv, Wo)
    return out


# revision 10
# speedup vs baseline: 1.1887x; 1.1512x over previous
"""Multi-head attention (L=2048, EMB=1024, H=16, D=64) on 8 TRN2 NeuronCores.

Tensor-parallel over heads: core i owns heads {2i, 2i+1} (a 128-row block of
Wq/Wk/Wv and a 128-column block of Wo). Each core computes its two heads'
attention plus its partial output projection; the host sums the 8 partials.

Device-side layout is fully transposed (scores^T = [m, l]) so no on-device
transposes are needed:
  QT[d, l] = (Wq_shard @ q^T)        lhsT = (Wq_shard/8)^T, rhs = q^T
  KT[d, l] = (Wk_shard @ k^T)
  V [m, d] = (v @ Wv_shard^T)        lhsT = v^T tile,       rhs = Wv_shard^T
  sT[m, l] = KT_h^T @ QT_h           (per head, contraction d=64)
  pT       = exp(sT) * keepT         (no max-subtraction: |s| <~ 9)
  attnT|Z  = [V_h | 1*64]^T @ pT     (ones cols 64:128 broadcast the softmax
                                      denominator Z onto PSUM rows 64:127)
  attnT/Z  = pa[0:64] * recip(pa[64:128])   local DVE, no DRAM bounce
  outT     = Wo_shard^T-block @ (attnT / Z)   bf16 partial, summed on host

All matmuls run in bf16 (fp32 PSUM accumulation); measured end-to-end
relative error vs the fp32 reference is ~0.6%.

Schedule notes (tuned against neuron-profile NTFF traces + the CoreSim
cost model):
- PE clock ramps 0.65 -> 1.2 -> 2.4 GHz with sustained use; every idle gap
  resets the ramp, so the whole schedule aims to keep the PE queue fed.
- The mask ships as fp8e4 (8 MB/core) and is upcast to bf16 in-flight by
  the gpsimd SWDGE DMA, so the DVE multiply keeps its 2x 16-bit rate.
- Output partials are stored bf16 (host sums in f64): halves store traffic.
- Critical-path input DMAs are split across rings: scalar gets {wq, q},
  sync gets {wk, k, wv, v, wo}, gpsimd gets mask fetches (half-tiles, one
  pass ahead), so the first scores matmul issues ~12us in.
- One-stage software pipeline on the PE queue: quad q's attn matmuls are
  emitted after quad q+1's scores, decoupling PE from the exp->mask-mult
  chain; exp is batched 3 key-tiles per ACTIVATE.
- Per-l-tile output projection is drip-fed one piece per quad into the
  next passes' streams as PE filler.
"""

import sys

for _p in ("/opt/trn_rl_repo",):
    if _p not in sys.path:
        sys.path.insert(0, _p)

from contextlib import ExitStack

import ml_dtypes
import numpy as np

import concourse.bass as bass
import concourse.tile as tile
from concourse import bacc, mybir
from concourse._compat import with_exitstack
from concourse.bass_utils import run_bass_kernel_spmd

BF16 = mybir.dt.bfloat16
FP8 = mybir.dt.float8e4
F32 = mybir.dt.float32
NPBF16 = ml_dtypes.bfloat16
NPFP8 = ml_dtypes.float8_e4m3

L = 2048
EMB = 1024
NHEAD = 16
HEAD_DIM = 64
NCORES = 8
HPC = NHEAD // NCORES  # heads per core = 2
ROWS = HPC * HEAD_DIM  # weight rows per core = 128
SCALE = HEAD_DIM ** -0.5

LT = 512               # l-tile (matmul free dim / PSUM bank)
NLT = L // LT          # 4
MT = 128               # m-tile (key-block on partitions)
NMT = L // MT          # 16
ET = 128               # contraction tile over EMB
NET = EMB // ET        # 8
JT = 128               # output-row tile
NJT = EMB // JT        # 8

QB = 2                 # psc tile m-capacity (PSUM banks per slot)
NSTEP = NMT // QB      # 8 mt-pair steps per (lt) with both heads interleaved
VROW = 128             # attn lhsT free dim: 64 V rows + 64 ones rows (Z bcast)


@with_exitstack
def _mha_kernel(ctx, tc, outT, qT, kT, vT, wqT, wkT, wvT, woT, maskT):
    nc = tc.nc

    const = ctx.enter_context(tc.tile_pool(name="const", bufs=1))
    ppool = ctx.enter_context(tc.tile_pool(name="ptiles", bufs=6))
    maskp = ctx.enter_context(tc.tile_pool(name="maskp", bufs=5))
    stage = ctx.enter_context(tc.tile_pool(name="stage", bufs=4))
    zpool = ctx.enter_context(tc.tile_pool(name="zpool", bufs=4))
    psc = ctx.enter_context(tc.tile_pool(name="psc", bufs=3, space="PSUM"))
    psa = ctx.enter_context(tc.tile_pool(name="psa", bufs=2, space="PSUM"))

    # ---- resident input tiles; DMAs split across rings in consumption order
    qTs = const.tile([128, NET, L], BF16, tag="qTs")
    kTs = const.tile([128, NET, L], BF16, tag="kTs")
    vTs = const.tile([128, NET, L], BF16, tag="vTs")
    wqs = const.tile([128, NET, ROWS], BF16, tag="wqs")
    wks = const.tile([128, NET, ROWS], BF16, tag="wks")
    wvs = const.tile([128, NET, ROWS], BF16, tag="wvs")
    wos = const.tile([128, EMB], BF16, tag="wos")  # [hd, j]
    q3 = qT.rearrange("(o p) l -> p o l", p=128)
    k3 = kT.rearrange("(o p) l -> p o l", p=128)
    v3 = vT.rearrange("(o p) l -> p o l", p=128)
    mask3 = maskT.rearrange("h (mo p) l -> h p mo l", p=128)
    out3 = outT.rearrange("(b p) l -> p b l", p=128)

    def chunk(eng, dst, src3, lc):
        eng.dma_start(dst[:, :, bass.ts(lc, LT)], src3[:, :, bass.ts(lc, LT)])

    # scalar ring: q-side critical path, then q tail; nothing else ever
    # (keeps the ACT engine free for exp)
    nc.scalar.dma_start(wqs[:], wqT[:])
    chunk(nc.scalar, qTs, q3, 0)
    for lc in range(1, NLT):
        chunk(nc.scalar, qTs, q3, lc)
    # sync ring: k/v-side critical path, then bulk
    nc.sync.dma_start(wks[:], wkT[:])
    chunk(nc.sync, kTs, k3, 0)
    nc.sync.dma_start(wvs[:], wvT[:])
    chunk(nc.sync, vTs, v3, 0)
    for lc in range(1, NLT):
        chunk(nc.sync, kTs, k3, lc)
        chunk(nc.sync, vTs, v3, lc)
    nc.sync.dma_start(wos[:], woT[:])

    state = {}

    def mask_fetch(lt, h, half):
        mc = maskp.tile([128, 8, LT], BF16, tag="maskc",
                        name=f"maskc_{lt}_{h}_{half}")
        nc.gpsimd.dma_start(
            mc[:], mask3[h, :, 8 * half : 8 * half + 8, bass.ts(lt, LT)]
        )
        state[lt, h, half] = mc

    mask_fetch(0, 0, 0)
    mask_fetch(0, 1, 0)
    mask_fetch(0, 0, 1)
    mask_fetch(0, 1, 1)

    QTb = const.tile([128, L], BF16, tag="QTb")
    KTb = const.tile([128, L], BF16, tag="KTb")
    vaug = const.tile([128, HPC, NMT, VROW], BF16, tag="vaug")
    nc.vector.memset(vaug[:, :, :, HEAD_DIM:VROW], 1.0)
    attnTb = const.tile([128, L], BF16, tag="attnTb")

    def qk_proj(dst, w, x, lt, ps, use_act):
        for et in range(NET):
            nc.tensor.matmul(
                ps[:],
                lhsT=w[:, et, :],
                rhs=x[:, et, bass.ts(lt, LT)],
                start=(et == 0),
                stop=(et == NET - 1),
            )
        if use_act:
            nc.scalar.copy(out=dst[:, bass.ts(lt, LT)], in_=ps[:])
        else:
            nc.vector.tensor_copy(out=dst[:, bass.ts(lt, LT)], in_=ps[:])

    # NOTE: PSUM accumulation groups are bank-granular; only one open group
    # per bank at a time (interleaving two in a bank corrupts both).
    def v_proj(mt):
        ps = psc.tile([128, QB, LT], F32, tag="psc", name="ps_v")[:, 0, :]
        for et in range(NET):
            nc.tensor.matmul(
                ps[:, :ROWS],
                lhsT=vTs[:, et, bass.ts(mt, MT)],
                rhs=wvs[:, et, :],
                start=(et == 0),
                stop=(et == NET - 1),
            )
        for h in range(HPC):
            nc.vector.tensor_copy(
                out=vaug[:, h, mt, 0:HEAD_DIM],
                in_=ps[:, bass.ts(h, HEAD_DIM)],
            )

    def k_proj(lc):
        ps = psc.tile([128, QB, LT], F32, tag="psc", name="ps_k")[:, 0, :]
        qk_proj(KTb, wks, kTs, lc, ps, use_act=True)

    def q_proj(lc):
        ps = psc.tile([128, QB, LT], F32, tag="psc", name="ps_q")[:, 0, :]
        qk_proj(QTb, wqs, qTs, lc, ps, use_act=False)

    # ---- prologue: Q(lt0) and K(chunk0) projections on separate psc slots
    ps_q0 = psc.tile([128, QB, LT], F32, tag="psc", name="ps_q0")[:, 0, :]
    qk_proj(QTb, wqs, qTs, 0, ps_q0, use_act=True)
    ps_k0 = psc.tile([128, QB, LT], F32, tag="psc", name="ps_k0")[:, 0, :]
    qk_proj(KTb, wks, kTs, 0, ps_k0, use_act=True)

    # lt0 PE filler by step: K chunk b feeds scores of step 2b; v(mt) feeds
    # the attn matmuls of step mt//2 which are emitted at step mt//2 + 1.
    lt0_filler = {
        0: [lambda: v_proj(0), lambda: v_proj(1)],
        1: [lambda: v_proj(2), lambda: v_proj(3)],
        2: [lambda: k_proj(1), lambda: v_proj(4), lambda: v_proj(5)],
        3: [lambda: v_proj(6), lambda: v_proj(7), lambda: q_proj(1)],
        4: [lambda: k_proj(2), lambda: v_proj(8), lambda: v_proj(9)],
        5: [lambda: v_proj(10), lambda: v_proj(11), lambda: q_proj(2)],
        6: [lambda: k_proj(3), lambda: v_proj(12), lambda: v_proj(13)],
        7: [lambda: v_proj(14), lambda: v_proj(15), lambda: q_proj(3)],
    }

    # deferred per-l-tile output projection (2 jt per psc slot, one per
    # bank — sequential groups in separate banks), drip-fed as PE filler
    pending = []

    def piece_outproj(lt, jp):
        def go():
            ls = bass.ts(lt, LT)
            ps = psc.tile([128, QB, LT], F32, tag="psc", name="ps_out")
            st = stage.tile([128, 2, LT], BF16, tag="st", name=f"st_{lt}_{jp}")
            for i in range(2):
                nc.tensor.matmul(
                    ps[:, i, :],
                    lhsT=wos[:, bass.ts(2 * jp + i, JT)],
                    rhs=attnTb[:, ls],
                    start=True,
                    stop=True,
                )
            nc.vector.tensor_copy(out=st[:, 0, :], in_=ps[:, 0, :])
            nc.scalar.copy(out=st[:, 1, :], in_=ps[:, 1, :])
            nc.sync.dma_start(out3[:, 2 * jp : 2 * jp + 2, ls], st[:])
        return go

    for lt in range(NLT):
        ls = bass.ts(lt, LT)
        pa = [psa.tile([128, LT], F32, tag="psa", name=f"psa_{lt}_{h}")
              for h in range(HPC)]
        prev_attn = [None, None]
        for s in range(NSTEP):
            mt0 = QB * s
            half = 0 if mt0 < 8 else 1
            j0 = mt0 - 8 * half
            if lt == 0:
                for f in lt0_filler.get(s, ()):
                    f()
            else:
                if pending and s % 2 == 1:
                    pending.pop(0)()
            # prefetch next lt's mask halves across steps 3..6
            if lt + 1 < NLT and 3 <= s <= 6:
                mask_fetch(lt + 1, (s - 3) % 2, (s - 3) // 2)
            for h in range(HPC):
                hd = bass.ts(h, HEAD_DIM)
                maskc = state[lt, h, half]
                ss = psc.tile([128, QB, LT], F32, tag="psc", name="ss")
                for i in range(QB):
                    nc.tensor.matmul(
                        ss[:, i, :],
                        lhsT=KTb[hd, bass.ts(mt0 + i, MT)],
                        rhs=QTb[hd, ls],
                        start=True,
                        stop=True,
                    )
                # software pipeline: this head's previous attn matmuls are
                # emitted after this step's scores
                if prev_attn[h] is not None:
                    prev_attn[h]()
                pT = ppool.tile([128, QB, LT], BF16, tag="pT", name="pT")
                nc.scalar.activation(
                    pT[:], ss[:], mybir.ActivationFunctionType.Exp
                )
                nc.vector.tensor_mul(
                    out=pT[:], in0=pT[:], in1=maskc[:, j0 : j0 + QB, :],
                )

                def make_attn(h=h, mt0=mt0, pT=pT):
                    def go():
                        for i in range(QB):
                            mt = mt0 + i
                            nc.tensor.matmul(
                                pa[h][:],
                                lhsT=vaug[:, h, mt, :],
                                rhs=pT[:, i, :],
                                start=(mt == 0),
                                stop=(mt == NMT - 1),
                            )
                    return go

                prev_attn[h] = make_attn()
        for h in range(HPC):
            prev_attn[h]()
        for h in range(HPC):
            hd = bass.ts(h, HEAD_DIM)
            # local softmax normalize: rows 64:127 of pa all hold Z
            zinv = zpool.tile([64, LT], F32, tag="zinv", name=f"zinv_{lt}_{h}")
            nc.vector.reciprocal(zinv[:], pa[h][64:128, :])
            nc.vector.tensor_mul(
                out=attnTb[hd, ls], in0=pa[h][0:HEAD_DIM, :], in1=zinv[:]
            )
        for jp in range(NJT // 2):
            pending.append(piece_outproj(lt, jp))

    while pending:
        pending.pop(0)()


_CACHE = {}


def _build():
    if "nc" in _CACHE:
        return _CACHE["nc"]
    nc = bacc.Bacc("TRN2", target_bir_lowering=False, debug=False,
                   num_devices=NCORES)
    qT = nc.dram_tensor("qT", [EMB, L], BF16, kind="ExternalInput").ap()
    kT = nc.dram_tensor("kT", [EMB, L], BF16, kind="ExternalInput").ap()
    vT = nc.dram_tensor("vT", [EMB, L], BF16, kind="ExternalInput").ap()
    wqT = nc.dram_tensor("wqT", [128, NET, ROWS], BF16, kind="ExternalInput").ap()
    wkT = nc.dram_tensor("wkT", [128, NET, ROWS], BF16, kind="ExternalInput").ap()
    wvT = nc.dram_tensor("wvT", [128, NET, ROWS], BF16, kind="ExternalInput").ap()
    woT = nc.dram_tensor("woT", [ROWS, EMB], BF16, kind="ExternalInput").ap()
    maskT = nc.dram_tensor("maskT", [HPC, L, L], FP8, kind="ExternalInput").ap()
    outT = nc.dram_tensor("outT", [EMB, L], BF16, kind="ExternalOutput").ap()

    with tile.TileContext(nc) as tc:
        _mha_kernel(tc, outT, qT, kT, vT, wqT, wkT, wvT, woT, maskT)
    nc.compile()
    _CACHE["nc"] = nc
    return nc


def _pack_w(w):
    # [ROWS, EMB] -> w.T [EMB, ROWS] -> [128, NET, ROWS] with e = o*128+p
    return np.ascontiguousarray(
        w.T.reshape(NET, 128, ROWS).transpose(1, 0, 2)
    ).astype(NPBF16)


def _prep_in_maps(q, k, v, mask, Wq, Wk, Wv, Wo):
    qT = np.ascontiguousarray(q.T).astype(NPBF16)
    kT = np.ascontiguousarray(k.T).astype(NPBF16)
    vT = np.ascontiguousarray(v.T).astype(NPBF16)
    in_maps = []
    for c in range(NCORES):
        rows = slice(c * ROWS, (c + 1) * ROWS)
        in_maps.append({
            "qT": qT,
            "kT": kT,
            "vT": vT,
            "wqT": _pack_w(Wq[rows] * SCALE),
            "wkT": _pack_w(Wk[rows]),
            "wvT": _pack_w(Wv[rows]),
            "woT": np.ascontiguousarray(Wo[:, rows].T).astype(NPBF16),
            "maskT": np.ascontiguousarray(
                (~mask[c * HPC : (c + 1) * HPC]).swapaxes(1, 2)
            ).astype(NPFP8),
        })
    return in_maps


def run(q, k, v, mask, Wq, Wk, Wv, Wo, **spmd_kwargs):
    nc = _build()
    in_maps = _prep_in_maps(q, k, v, mask, Wq, Wk, Wv, Wo)
    res = run_bass_kernel_spmd(nc, in_maps, list(range(NCORES)), **spmd_kwargs)
    outT = np.zeros((EMB, L), np.float64)
    for r in res.results:
        outT += np.asarray(r["outT"]).astype(np.float64)
    out = np.ascontiguousarray(outT.T).astype(np.float32)
    return out, res


def kernel(q, k, v, mask, Wq, Wk, Wv, Wo):
    q, k, v = (np.asarray(x, np.float32) for x in (q, k, v))
    Wq, Wk, Wv, Wo = (np.asarray(x, np.float32) for x in (Wq, Wk, Wv, Wo))
    mask = np.asarray(mask, bool)
    out, _ = run(q, k, v, mask, Wq, Wk, Wv, Wo)
    return out


# revision 12
# speedup vs baseline: 1.3875x; 1.1672x over previous
"""Multi-head attention (L=2048, EMB=1024, H=16, D=64) on 8 TRN2 NeuronCores.

Tensor-parallel over heads: core i owns heads {2i, 2i+1} (a 128-row block of
Wq/Wk/Wv and a 128-column block of Wo). Each core computes its two heads'
attention plus its partial output projection; the host sums the 8 partials.

Device-side layout is fully transposed (scores^T = [m, l]) so no on-device
transposes are needed:
  QT[d, l] = (Wq_shard @ q^T)        lhsT = (Wq_shard/8)^T, rhs = q^T
  KT[d, l] = (Wk_shard @ k^T)
  V [m, d] = (v @ Wv_shard^T)        lhsT = v^T tile,       rhs = Wv_shard^T
  sT[m, l] = KT_h^T @ QT_h           (per head, contraction d=64)
  pT       = exp(sT) * keepT         (no max-subtraction: |s| <~ 9)
  attnT|Z  = [V_h | 1*64]^T @ pT     (ones cols 64:128 broadcast the softmax
                                      denominator Z onto PSUM rows 64:127)
  attnT/Z  = pa[0:64] * recip(pa[64:128])   local DVE, no DRAM bounce
  outT     = Wo_shard^T-block @ (attnT / Z)   bf16 partial, summed on host

All matmuls run in bf16 (fp32 PSUM accumulation); measured end-to-end
relative error vs the fp32 reference is ~0.6%.

Schedule notes (tuned against neuron-profile NTFF traces + the CoreSim
cost model):
- PE clock ramps 0.65 -> 1.2 -> 2.4 GHz with sustained use; every idle gap
  resets the ramp, so the whole schedule aims to keep the PE queue fed.
- The mask ships as fp8e4 (8 MB/core) and is upcast to bf16 in-flight by
  the gpsimd SWDGE DMA, so the DVE multiply keeps its 2x 16-bit rate.
- Output partials are stored bf16 (host sums in f64): halves store traffic.
- Critical-path input DMAs are split across rings: scalar gets {wq, q},
  sync gets {wk, k, wv, v, wo}, gpsimd gets mask fetches (half-tiles, one
  pass ahead), so the first scores matmul issues ~12us in.
- One-stage software pipeline on the PE queue: quad q's attn matmuls are
  emitted after quad q+1's scores, decoupling PE from the exp->mask-mult
  chain; exp is batched 3 key-tiles per ACTIVATE.
- Per-l-tile output projection is drip-fed one piece per quad into the
  next passes' streams as PE filler.
"""

import sys

for _p in ("/opt/trn_rl_repo",):
    if _p not in sys.path:
        sys.path.insert(0, _p)

from contextlib import ExitStack

import ml_dtypes
import numpy as np

import concourse.bass as bass
import concourse.tile as tile
from concourse import bacc, mybir
from concourse._compat import with_exitstack
from concourse.bass_utils import run_bass_kernel_spmd

BF16 = mybir.dt.bfloat16
FP8 = mybir.dt.float8e4
F32 = mybir.dt.float32
NPBF16 = ml_dtypes.bfloat16
NPFP8 = ml_dtypes.float8_e4m3

L = 2048
EMB = 1024
NHEAD = 16
HEAD_DIM = 64
NCORES = 8
HPC = NHEAD // NCORES  # heads per core = 2
ROWS = HPC * HEAD_DIM  # weight rows per core = 128
SCALE = HEAD_DIM ** -0.5

LT = 512               # l-tile (matmul free dim / PSUM bank)
NLT = L // LT          # 4
MT = 128               # m-tile (key-block on partitions)
NMT = L // MT          # 16
ET = 128               # contraction tile over EMB
NET = EMB // ET        # 8
JT = 128               # output-row tile
NJT = EMB // JT        # 8

QB = 2                 # psc tile m-capacity (PSUM banks per slot)
NSTEP = NMT // QB      # 8 mt-pair steps per (lt) with both heads interleaved
VROW = 128             # attn lhsT free dim: 64 V rows + 64 ones rows (Z bcast)


@with_exitstack
def _mha_kernel(ctx, tc, outT, qT, kT, vT, wqT, wkT, wvT, woT, maskT):
    nc = tc.nc

    const = ctx.enter_context(tc.tile_pool(name="const", bufs=1))
    ppool = ctx.enter_context(tc.tile_pool(name="ptiles", bufs=6))
    maskp = ctx.enter_context(tc.tile_pool(name="maskp", bufs=5))
    stage = ctx.enter_context(tc.tile_pool(name="stage", bufs=4))
    zpool = ctx.enter_context(tc.tile_pool(name="zpool", bufs=4))
    psc = ctx.enter_context(tc.tile_pool(name="psc", bufs=3, space="PSUM"))
    psa = ctx.enter_context(tc.tile_pool(name="psa", bufs=2, space="PSUM"))

    # ---- resident input tiles; DMAs split across rings in consumption order
    qTs = const.tile([128, NET, L], BF16, tag="qTs")
    kTs = const.tile([128, NET, L], BF16, tag="kTs")
    vTs = const.tile([128, NET, L], BF16, tag="vTs")
    wqs = const.tile([128, NET, ROWS], BF16, tag="wqs")
    wks = const.tile([128, NET, ROWS], BF16, tag="wks")
    wvs = const.tile([128, NET, ROWS], BF16, tag="wvs")
    wos = const.tile([128, EMB], BF16, tag="wos")  # [hd, j]
    q3 = qT.rearrange("(o p) l -> p o l", p=128)
    k3 = kT.rearrange("(o p) l -> p o l", p=128)
    v3 = vT.rearrange("(o p) l -> p o l", p=128)
    mask3 = maskT.rearrange("h (mo p) l -> h p mo l", p=128)
    out3 = outT.rearrange("(b p) l -> p b l", p=128)

    def chunk(eng, dst, src3, lc):
        eng.dma_start(dst[:, :, bass.ts(lc, LT)], src3[:, :, bass.ts(lc, LT)])

    # scalar ring: q-side critical path, then q tail; nothing else ever
    # (keeps the ACT engine free for exp)
    nc.scalar.dma_start(wqs[:], wqT[:])
    chunk(nc.scalar, qTs, q3, 0)
    for lc in range(1, NLT):
        chunk(nc.scalar, qTs, q3, lc)
    # sync ring: k/v-side critical path, then bulk
    nc.sync.dma_start(wks[:], wkT[:])
    chunk(nc.sync, kTs, k3, 0)
    nc.sync.dma_start(wvs[:], wvT[:])
    chunk(nc.sync, vTs, v3, 0)
    for lc in range(1, NLT):
        chunk(nc.sync, kTs, k3, lc)
        chunk(nc.sync, vTs, v3, lc)
    nc.sync.dma_start(wos[:], woT[:])

    state = {}

    def mask_fetch(lt, h, half):
        mc = maskp.tile([128, 8, LT], BF16, tag="maskc",
                        name=f"maskc_{lt}_{h}_{half}")
        nc.gpsimd.dma_start(
            mc[:], mask3[h, :, 8 * half : 8 * half + 8, bass.ts(lt, LT)]
        )
        state[lt, h, half] = mc

    mask_fetch(0, 0, 0)
    mask_fetch(0, 1, 0)
    mask_fetch(0, 0, 1)
    mask_fetch(0, 1, 1)

    QTb = const.tile([128, L], BF16, tag="QTb")
    KTb = const.tile([128, L], BF16, tag="KTb")
    vaug = const.tile([128, HPC, NMT, VROW], BF16, tag="vaug")
    nc.vector.memset(vaug[:, :, :, HEAD_DIM:VROW], 1.0)
    attnTb = const.tile([128, L], BF16, tag="attnTb")

    def qk_proj(dst, w, x, lt, ps, use_act):
        for et in range(NET):
            nc.tensor.matmul(
                ps[:],
                lhsT=w[:, et, :],
                rhs=x[:, et, bass.ts(lt, LT)],
                start=(et == 0),
                stop=(et == NET - 1),
            )
        if use_act:
            nc.scalar.copy(out=dst[:, bass.ts(lt, LT)], in_=ps[:])
        else:
            nc.vector.tensor_copy(out=dst[:, bass.ts(lt, LT)], in_=ps[:])

    # NOTE: PSUM accumulation groups are bank-granular; only one open group
    # per bank at a time (interleaving two in a bank corrupts both).
    def v_proj(mt):
        ps = psc.tile([128, QB, LT], F32, tag="psc", name="ps_v")[:, 0, :]
        for et in range(NET):
            nc.tensor.matmul(
                ps[:, :ROWS],
                lhsT=vTs[:, et, bass.ts(mt, MT)],
                rhs=wvs[:, et, :],
                start=(et == 0),
                stop=(et == NET - 1),
            )
        for h in range(HPC):
            nc.vector.tensor_copy(
                out=vaug[:, h, mt, 0:HEAD_DIM],
                in_=ps[:, bass.ts(h, HEAD_DIM)],
            )

    def k_proj(lc):
        ps = psc.tile([128, QB, LT], F32, tag="psc", name="ps_k")[:, 0, :]
        qk_proj(KTb, wks, kTs, lc, ps, use_act=True)

    def q_proj(lc):
        ps = psc.tile([128, QB, LT], F32, tag="psc", name="ps_q")[:, 0, :]
        qk_proj(QTb, wqs, qTs, lc, ps, use_act=False)

    # ---- prologue: Q(lt0) and K(chunk0) projections on separate psc slots
    ps_q0 = psc.tile([128, QB, LT], F32, tag="psc", name="ps_q0")[:, 0, :]
    qk_proj(QTb, wqs, qTs, 0, ps_q0, use_act=True)
    ps_k0 = psc.tile([128, QB, LT], F32, tag="psc", name="ps_k0")[:, 0, :]
    qk_proj(KTb, wks, kTs, 0, ps_k0, use_act=True)

    # lt0 PE filler by step: K chunk b feeds scores of step 2b; v(mt) feeds
    # the attn matmuls of step mt//2 which are emitted at step mt//2 + 1.
    lt0_filler = {
        0: [lambda: v_proj(0), lambda: v_proj(1)],
        1: [lambda: v_proj(2), lambda: v_proj(3)],
        2: [lambda: k_proj(1), lambda: v_proj(4), lambda: v_proj(5)],
        3: [lambda: v_proj(6), lambda: v_proj(7), lambda: q_proj(1)],
        4: [lambda: k_proj(2), lambda: v_proj(8), lambda: v_proj(9)],
        5: [lambda: v_proj(10), lambda: v_proj(11), lambda: q_proj(2)],
        6: [lambda: k_proj(3), lambda: v_proj(12), lambda: v_proj(13)],
        7: [lambda: v_proj(14), lambda: v_proj(15), lambda: q_proj(3)],
    }

    # deferred per-l-tile output projection (2 jt per psc slot, one per
    # bank — sequential groups in separate banks), drip-fed as PE filler
    pending = []

    def piece_outproj(lt, jp):
        def go():
            ls = bass.ts(lt, LT)
            ps = psc.tile([128, QB, LT], F32, tag="psc", name="ps_out")
            st = stage.tile([128, 2, LT], BF16, tag="st", name=f"st_{lt}_{jp}")
            for i in range(2):
                nc.tensor.matmul(
                    ps[:, i, :],
                    lhsT=wos[:, bass.ts(2 * jp + i, JT)],
                    rhs=attnTb[:, ls],
                    start=True,
                    stop=True,
                )
            nc.vector.tensor_copy(out=st[:, 0, :], in_=ps[:, 0, :])
            nc.scalar.copy(out=st[:, 1, :], in_=ps[:, 1, :])
            nc.sync.dma_start(out3[:, 2 * jp : 2 * jp + 2, ls], st[:])
        return go

    for lt in range(NLT):
        ls = bass.ts(lt, LT)
        pa = [psa.tile([128, LT], F32, tag="psa", name=f"psa_{lt}_{h}")
              for h in range(HPC)]
        prev_attn = [None, None]
        for s in range(NSTEP):
            mt0 = QB * s
            half = 0 if mt0 < 8 else 1
            j0 = mt0 - 8 * half
            if lt == 0:
                for f in lt0_filler.get(s, ()):
                    f()
            else:
                if pending and s % 2 == 1:
                    pending.pop(0)()
            # prefetch next lt's mask halves across steps 3..6
            if lt + 1 < NLT and 3 <= s <= 6:
                mask_fetch(lt + 1, (s - 3) % 2, (s - 3) // 2)
            for h in range(HPC):
                hd = bass.ts(h, HEAD_DIM)
                maskc = state[lt, h, half]
                ss = psc.tile([128, QB, LT], F32, tag="psc", name="ss")
                for i in range(QB):
                    nc.tensor.matmul(
                        ss[:, i, :],
                        lhsT=KTb[hd, bass.ts(mt0 + i, MT)],
                        rhs=QTb[hd, ls],
                        start=True,
                        stop=True,
                    )
                # software pipeline: this head's previous attn matmuls are
                # emitted after this step's scores
                if prev_attn[h] is not None:
                    prev_attn[h]()
                pT = ppool.tile([128, QB, LT], BF16, tag="pT", name="pT")
                nc.scalar.activation(
                    pT[:], ss[:], mybir.ActivationFunctionType.Exp
                )
                nc.vector.tensor_mul(
                    out=pT[:], in0=pT[:], in1=maskc[:, j0 : j0 + QB, :],
                )

                def make_attn(h=h, mt0=mt0, pT=pT):
                    def go():
                        for i in range(QB):
                            mt = mt0 + i
                            nc.tensor.matmul(
                                pa[h][:],
                                lhsT=vaug[:, h, mt, :],
                                rhs=pT[:, i, :],
                                start=(mt == 0),
                                stop=(mt == NMT - 1),
                            )
                    return go

                prev_attn[h] = make_attn()
        for h in range(HPC):
            prev_attn[h]()
            hd = bass.ts(h, HEAD_DIM)
            # local softmax normalize: rows 64:127 of pa all hold Z
            # (emitted before the other head's last attn so DVE overlaps PE)
            # custom-DVE bitwise ops misread PSUM: stage Z to SBUF (ACT copy)
            # before the approx reciprocal
            zsb = zpool.tile([64, LT], F32, tag="zsb", name=f"zsb_{lt}_{h}")
            nc.scalar.copy(out=zsb[:], in_=pa[h][64:128, :])
            zinv = zpool.tile([64, LT], F32, tag="zinv", name=f"zinv_{lt}_{h}")
            nc.vector.reciprocal_approx_fast(out=zinv[:], in_=zsb[:])
            nc.vector.tensor_mul(
                out=attnTb[hd, ls], in0=pa[h][0:HEAD_DIM, :], in1=zinv[:]
            )
        for jp in range(NJT // 2):
            pending.append(piece_outproj(lt, jp))

    while pending:
        pending.pop(0)()


_CACHE = {}


def _build():
    if "nc" in _CACHE:
        return _CACHE["nc"]
    nc = bacc.Bacc("TRN2", target_bir_lowering=False, debug=False,
                   num_devices=NCORES)
    qT = nc.dram_tensor("qT", [EMB, L], BF16, kind="ExternalInput").ap()
    kT = nc.dram_tensor("kT", [EMB, L], BF16, kind="ExternalInput").ap()
    vT = nc.dram_tensor("vT", [EMB, L], BF16, kind="ExternalInput").ap()
    wqT = nc.dram_tensor("wqT", [128, NET, ROWS], BF16, kind="ExternalInput").ap()
    wkT = nc.dram_tensor("wkT", [128, NET, ROWS], BF16, kind="ExternalInput").ap()
    wvT = nc.dram_tensor("wvT", [128, NET, ROWS], BF16, kind="ExternalInput").ap()
    woT = nc.dram_tensor("woT", [ROWS, EMB], BF16, kind="ExternalInput").ap()
    maskT = nc.dram_tensor("maskT", [HPC, L, L], FP8, kind="ExternalInput").ap()
    outT = nc.dram_tensor("outT", [EMB, L], BF16, kind="ExternalOutput").ap()

    with tile.TileContext(nc) as tc:
        _mha_kernel(tc, outT, qT, kT, vT, wqT, wkT, wvT, woT, maskT)
    nc.compile()
    _CACHE["nc"] = nc
    return nc


def _pack_w(w):
    # [ROWS, EMB] -> w.T [EMB, ROWS] -> [128, NET, ROWS] with e = o*128+p
    return np.ascontiguousarray(
        w.T.reshape(NET, 128, ROWS).transpose(1, 0, 2)
    ).astype(NPBF16)


def _prep_in_maps(q, k, v, mask, Wq, Wk, Wv, Wo):
    qT = np.ascontiguousarray(q.T).astype(NPBF16)
    kT = np.ascontiguousarray(k.T).astype(NPBF16)
    vT = np.ascontiguousarray(v.T).astype(NPBF16)
    in_maps = []
    for c in range(NCORES):
        rows = slice(c * ROWS, (c + 1) * ROWS)
        in_maps.append({
            "qT": qT,
            "kT": kT,
            "vT": vT,
            "wqT": _pack_w(Wq[rows] * SCALE),
            "wkT": _pack_w(Wk[rows]),
            "wvT": _pack_w(Wv[rows]),
            "woT": np.ascontiguousarray(Wo[:, rows].T).astype(NPBF16),
            "maskT": np.ascontiguousarray(
                (~mask[c * HPC : (c + 1) * HPC]).swapaxes(1, 2)
            ).astype(NPFP8),
        })
    return in_maps


def run(q, k, v, mask, Wq, Wk, Wv, Wo, **spmd_kwargs):
    nc = _build()
    in_maps = _prep_in_maps(q, k, v, mask, Wq, Wk, Wv, Wo)
    res = run_bass_kernel_spmd(nc, in_maps, list(range(NCORES)), **spmd_kwargs)
    outT = np.zeros((EMB, L), np.float64)
    for r in res.results:
        outT += np.asarray(r["outT"]).astype(np.float64)
    out = np.ascontiguousarray(outT.T).astype(np.float32)
    return out, res


def kernel(q, k, v, mask, Wq, Wk, Wv, Wo):
    q, k, v = (np.asarray(x, np.float32) for x in (q, k, v))
    Wq, Wk, Wv, Wo = (np.asarray(x, np.float32) for x in (Wq, Wk, Wv, Wo))
    mask = np.asarray(mask, bool)
    out, _ = run(q, k, v, mask, Wq, Wk, Wv, Wo)
    return out


# revision 16
# speedup vs baseline: 1.3970x; 1.0069x over previous
"""Multi-head attention (L=2048, EMB=1024, H=16, D=64) on 8 TRN2 NeuronCores.

Tensor-parallel over heads: core i owns heads {2i, 2i+1} (a 128-row block of
Wq/Wk/Wv and a 128-column block of Wo). Each core computes its two heads'
attention plus its partial output projection; the host sums the 8 partials.

Device-side layout is fully transposed (scores^T = [m, l]) so no on-device
transposes are needed:
  QT[d, l] = (Wq_shard @ q^T)        lhsT = (Wq_shard/8)^T, rhs = q^T
  KT[d, l] = (Wk_shard @ k^T)
  V [m, d] = (v @ Wv_shard^T)        lhsT = v^T tile,       rhs = Wv_shard^T
  sT[m, l] = KT_h^T @ QT_h           (per head, contraction d=64)
  pT       = exp(sT) * keepT         (no max-subtraction: |s| <~ 9)
  attnT|Z  = [V_h | 1*64]^T @ pT     (ones cols 64:128 broadcast the softmax
                                      denominator Z onto PSUM rows 64:127)
  attnT/Z  = pa[0:64] * recip(pa[64:128])   local DVE, no DRAM bounce
  outT     = Wo_shard^T-block @ (attnT / Z)   bf16 partial, summed on host

All matmuls run in bf16 (fp32 PSUM accumulation); measured end-to-end
relative error vs the fp32 reference is ~0.6%.

Schedule notes (tuned against neuron-profile NTFF traces + the CoreSim
cost model):
- PE clock ramps 0.65 -> 1.2 -> 2.4 GHz with sustained use; every idle gap
  resets the ramp, so the whole schedule aims to keep the PE queue fed.
- The mask ships as fp8e4 (8 MB/core) and is upcast to bf16 in-flight by
  the gpsimd SWDGE DMA, so the DVE multiply keeps its 2x 16-bit rate.
- Output partials are stored bf16 (host sums in f64): halves store traffic.
- Critical-path input DMAs are split across rings: scalar gets {wq, q},
  sync gets {wk, k, wv, v, wo}, gpsimd gets mask fetches (half-tiles, one
  pass ahead), so the first scores matmul issues ~12us in.
- One-stage software pipeline on the PE queue: quad q's attn matmuls are
  emitted after quad q+1's scores, decoupling PE from the exp->mask-mult
  chain; exp is batched 3 key-tiles per ACTIVATE.
- Per-l-tile output projection is drip-fed one piece per quad into the
  next passes' streams as PE filler.
"""

import sys

for _p in ("/opt/trn_rl_repo",):
    if _p not in sys.path:
        sys.path.insert(0, _p)

from contextlib import ExitStack

import ml_dtypes
import numpy as np

import concourse.bass as bass
import concourse.tile as tile
from concourse import bacc, mybir
from concourse._compat import with_exitstack
from concourse.bass_utils import run_bass_kernel_spmd

BF16 = mybir.dt.bfloat16
FP8 = mybir.dt.float8e4
F32 = mybir.dt.float32
NPBF16 = ml_dtypes.bfloat16
NPFP8 = ml_dtypes.float8_e4m3

L = 2048
EMB = 1024
NHEAD = 16
HEAD_DIM = 64
NCORES = 8
HPC = NHEAD // NCORES  # heads per core = 2
ROWS = HPC * HEAD_DIM  # weight rows per core = 128
SCALE = HEAD_DIM ** -0.5

LT = 512               # l-tile (matmul free dim / PSUM bank)
NLT = L // LT          # 4
MT = 128               # m-tile (key-block on partitions)
NMT = L // MT          # 16
ET = 128               # contraction tile over EMB
NET = EMB // ET        # 8
JT = 128               # output-row tile
NJT = EMB // JT        # 8

QB = 2                 # psc tile m-capacity (PSUM banks per slot)
NSTEP = NMT // QB      # 8 mt-pair steps per (lt) with both heads interleaved
VROW = 128             # attn lhsT free dim: 64 V rows + 64 ones rows (Z bcast)


@with_exitstack
def _mha_kernel(ctx, tc, outT, qT, kT, vT, wqT, wkT, wvT, woT, maskT):
    nc = tc.nc

    const = ctx.enter_context(tc.tile_pool(name="const", bufs=1))
    ppool = ctx.enter_context(tc.tile_pool(name="ptiles", bufs=6))
    maskp = ctx.enter_context(tc.tile_pool(name="maskp", bufs=5))
    stage = ctx.enter_context(tc.tile_pool(name="stage", bufs=4))
    zpool = ctx.enter_context(tc.tile_pool(name="zpool", bufs=4))
    psc = ctx.enter_context(tc.tile_pool(name="psc", bufs=3, space="PSUM"))
    psa = ctx.enter_context(tc.tile_pool(name="psa", bufs=2, space="PSUM"))

    # ---- resident input tiles; DMAs split across rings in consumption order
    qTs = const.tile([128, NET, L], BF16, tag="qTs")
    kTs = const.tile([128, NET, L], BF16, tag="kTs")
    vTs = const.tile([128, NET, L], BF16, tag="vTs")
    wqs = const.tile([128, NET, ROWS], BF16, tag="wqs")
    wks = const.tile([128, NET, ROWS], BF16, tag="wks")
    wvs = const.tile([128, NET, ROWS], BF16, tag="wvs")
    wos = const.tile([128, EMB], BF16, tag="wos")  # [hd, j]
    q3 = qT.rearrange("(o p) l -> p o l", p=128)
    k3 = kT.rearrange("(o p) l -> p o l", p=128)
    v3 = vT.rearrange("(o p) l -> p o l", p=128)
    mask3 = maskT.rearrange("h (mo p) l -> h p mo l", p=128)
    out3 = outT.rearrange("(b p) l -> p b l", p=128)

    def chunk(eng, dst, src3, lc):
        eng.dma_start(dst[:, :, bass.ts(lc, LT)], src3[:, :, bass.ts(lc, LT)])

    # scalar ring: q-side critical path, then q tail; nothing else ever
    # (keeps the ACT engine free for exp). q0/k0 split in et-halves so the
    # prologue projections pipeline with their own loads.
    nc.scalar.dma_start(wqs[:], wqT[:])
    nc.scalar.dma_start(qTs[:, 0:4, 0:LT], q3[:, 0:4, 0:LT])
    nc.scalar.dma_start(qTs[:, 4:8, 0:LT], q3[:, 4:8, 0:LT])
    for lc in range(1, NLT):
        chunk(nc.scalar, qTs, q3, lc)
    # sync ring: k/v-side critical path, then bulk
    nc.sync.dma_start(wks[:], wkT[:])
    nc.sync.dma_start(kTs[:, 0:4, 0:LT], k3[:, 0:4, 0:LT])
    nc.sync.dma_start(kTs[:, 4:8, 0:LT], k3[:, 4:8, 0:LT])
    nc.sync.dma_start(wvs[:], wvT[:])
    chunk(nc.sync, vTs, v3, 0)
    for lc in range(1, NLT):
        chunk(nc.sync, kTs, k3, lc)
        chunk(nc.sync, vTs, v3, lc)
    nc.sync.dma_start(wos[:], woT[:])

    state = {}

    def mask_fetch(lt, h, half):
        mc = maskp.tile([128, 8, LT], BF16, tag="maskc",
                        name=f"maskc_{lt}_{h}_{half}")
        nc.gpsimd.dma_start(
            mc[:], mask3[h, :, 8 * half : 8 * half + 8, bass.ts(lt, LT)]
        )
        state[lt, h, half] = mc

    mask_fetch(0, 0, 0)
    mask_fetch(0, 1, 0)
    mask_fetch(0, 0, 1)
    mask_fetch(0, 1, 1)

    QTb = const.tile([128, L], BF16, tag="QTb")
    KTb = const.tile([128, L], BF16, tag="KTb")
    vaug = const.tile([128, HPC, NMT, VROW], BF16, tag="vaug")
    nc.vector.memset(vaug[:, :, :, HEAD_DIM:VROW], 1.0)
    attnTb = const.tile([128, L], BF16, tag="attnTb")

    def qk_proj(dst, w, x, lt, ps, use_act):
        for et in range(NET):
            nc.tensor.matmul(
                ps[:],
                lhsT=w[:, et, :],
                rhs=x[:, et, bass.ts(lt, LT)],
                start=(et == 0),
                stop=(et == NET - 1),
            )
        if use_act:
            nc.scalar.copy(out=dst[:, bass.ts(lt, LT)], in_=ps[:])
        else:
            nc.vector.tensor_copy(out=dst[:, bass.ts(lt, LT)], in_=ps[:])

    # NOTE: PSUM accumulation groups are bank-granular; only one open group
    # per bank at a time (interleaving two in a bank corrupts both).
    def v_proj(mt):
        ps = psc.tile([128, QB, LT], F32, tag="psc", name="ps_v")[:, 0, :]
        for et in range(NET):
            nc.tensor.matmul(
                ps[:, :ROWS],
                lhsT=vTs[:, et, bass.ts(mt, MT)],
                rhs=wvs[:, et, :],
                start=(et == 0),
                stop=(et == NET - 1),
            )
        for h in range(HPC):
            nc.vector.tensor_copy(
                out=vaug[:, h, mt, 0:HEAD_DIM],
                in_=ps[:, bass.ts(h, HEAD_DIM)],
            )

    def k_proj(lc):
        ps = psc.tile([128, QB, LT], F32, tag="psc", name="ps_k")[:, 0, :]
        qk_proj(KTb, wks, kTs, lc, ps, use_act=True)

    def q_proj(lc):
        ps = psc.tile([128, QB, LT], F32, tag="psc", name="ps_q")[:, 0, :]
        qk_proj(QTb, wqs, qTs, lc, ps, use_act=False)

    # ---- prologue: Q(lt0) and K(chunk0) projections on separate psc slots,
    # emitted in et-half blocks interleaved in DMA-arrival order
    ps_q0 = psc.tile([128, QB, LT], F32, tag="psc", name="ps_q0")[:, 0, :]
    ps_k0 = psc.tile([128, QB, LT], F32, tag="psc", name="ps_k0")[:, 0, :]
    for lo, hi in ((0, 4), (4, 8)):
        for ps, w, x in ((ps_q0, wqs, qTs), (ps_k0, wks, kTs)):
            for et in range(lo, hi):
                nc.tensor.matmul(
                    ps[:],
                    lhsT=w[:, et, :],
                    rhs=x[:, et, 0:LT],
                    start=(et == 0),
                    stop=(et == NET - 1),
                )
    nc.scalar.copy(out=QTb[:, 0:LT], in_=ps_q0[:])
    nc.scalar.copy(out=KTb[:, 0:LT], in_=ps_k0[:])

    # lt0 PE filler by step: K chunk b feeds scores of step 2b; v(mt) feeds
    # the attn matmuls of step mt//2 which are emitted at step mt//2 + 1.
    lt0_filler = {
        0: [lambda: v_proj(0), lambda: v_proj(1)],
        1: [lambda: v_proj(2), lambda: v_proj(3)],
        2: [lambda: k_proj(1), lambda: v_proj(4), lambda: v_proj(5)],
        3: [lambda: v_proj(6), lambda: v_proj(7), lambda: q_proj(1)],
        4: [lambda: k_proj(2), lambda: v_proj(8), lambda: v_proj(9)],
        5: [lambda: v_proj(10), lambda: v_proj(11), lambda: q_proj(2)],
        6: [lambda: k_proj(3), lambda: v_proj(12), lambda: v_proj(13)],
        7: [lambda: v_proj(14), lambda: v_proj(15), lambda: q_proj(3)],
    }

    # deferred per-l-tile output projection (2 jt per psc slot, one per
    # bank — sequential groups in separate banks), drip-fed as PE filler
    pending = []

    def piece_outproj(lt, jp):
        def go():
            ls = bass.ts(lt, LT)
            ps = psc.tile([128, QB, LT], F32, tag="psc", name="ps_out")
            st = stage.tile([128, 2, LT], BF16, tag="st", name=f"st_{lt}_{jp}")
            for i in range(2):
                nc.tensor.matmul(
                    ps[:, i, :],
                    lhsT=wos[:, bass.ts(2 * jp + i, JT)],
                    rhs=attnTb[:, ls],
                    start=True,
                    stop=True,
                )
            nc.vector.tensor_copy(out=st[:, 0, :], in_=ps[:, 0, :])
            nc.scalar.copy(out=st[:, 1, :], in_=ps[:, 1, :])
            nc.sync.dma_start(out3[:, 2 * jp : 2 * jp + 2, ls], st[:])
        return go

    for lt in range(NLT):
        ls = bass.ts(lt, LT)
        pa = [psa.tile([128, LT], F32, tag="psa", name=f"psa_{lt}_{h}")
              for h in range(HPC)]
        prev_attn = [None, None]
        for s in range(NSTEP):
            mt0 = QB * s
            half = 0 if mt0 < 8 else 1
            j0 = mt0 - 8 * half
            if lt == 0:
                for f in lt0_filler.get(s, ()):
                    f()
            # prefetch next lt's mask halves across steps 3..6
            if lt + 1 < NLT and 3 <= s <= 6:
                mask_fetch(lt + 1, (s - 3) % 2, (s - 3) // 2)
            for h in range(HPC):
                hd = bass.ts(h, HEAD_DIM)
                maskc = state[lt, h, half]
                ss = psc.tile([128, QB, LT], F32, tag="psc", name="ss")
                for i in range(QB):
                    nc.tensor.matmul(
                        ss[:, i, :],
                        lhsT=KTb[hd, bass.ts(mt0 + i, MT)],
                        rhs=QTb[hd, ls],
                        start=True,
                        stop=True,
                    )
                # software pipeline: this head's previous attn matmuls are
                # emitted after this step's scores
                if prev_attn[h] is not None:
                    prev_attn[h]()
                pT = ppool.tile([128, QB, LT], BF16, tag="pT", name="pT")
                nc.scalar.activation(
                    pT[:], ss[:], mybir.ActivationFunctionType.Exp
                )
                nc.vector.tensor_mul(
                    out=pT[:], in0=pT[:], in1=maskc[:, j0 : j0 + QB, :],
                )

                def make_attn(h=h, mt0=mt0, pT=pT):
                    def go():
                        for i in range(QB):
                            mt = mt0 + i
                            nc.tensor.matmul(
                                pa[h][:],
                                lhsT=vaug[:, h, mt, :],
                                rhs=pT[:, i, :],
                                start=(mt == 0),
                                stop=(mt == NMT - 1),
                            )
                    return go

                prev_attn[h] = make_attn()
            # drip one deferred out-proj piece at the END of mid/late steps:
            # gives the previous lt's normalize chain slack so the piece's
            # matmuls (which read attnTb[prev]) don't stall the PE queue
            if pending and s in (2, 4, 6, 7):
                pending.pop(0)()
        for h in range(HPC):
            prev_attn[h]()
            hd = bass.ts(h, HEAD_DIM)
            # local softmax normalize: rows 64:127 of pa all hold Z
            # (emitted before the other head's last attn so DVE overlaps PE)
            # custom-DVE bitwise ops misread PSUM: stage Z to SBUF (ACT copy)
            # before the approx reciprocal
            zsb = zpool.tile([64, LT], F32, tag="zsb", name=f"zsb_{lt}_{h}")
            nc.scalar.copy(out=zsb[:], in_=pa[h][64:128, :])
            zinv = zpool.tile([64, LT], F32, tag="zinv", name=f"zinv_{lt}_{h}")
            nc.vector.reciprocal_approx_fast(out=zinv[:], in_=zsb[:])
            nc.vector.tensor_mul(
                out=attnTb[hd, ls], in0=pa[h][0:HEAD_DIM, :], in1=zinv[:]
            )
        for jp in range(NJT // 2):
            pending.append(piece_outproj(lt, jp))

    while pending:
        pending.pop(0)()


_CACHE = {}


def _build():
    if "nc" in _CACHE:
        return _CACHE["nc"]
    nc = bacc.Bacc("TRN2", target_bir_lowering=False, debug=False,
                   num_devices=NCORES)
    qT = nc.dram_tensor("qT", [EMB, L], BF16, kind="ExternalInput").ap()
    kT = nc.dram_tensor("kT", [EMB, L], BF16, kind="ExternalInput").ap()
    vT = nc.dram_tensor("vT", [EMB, L], BF16, kind="ExternalInput").ap()
    wqT = nc.dram_tensor("wqT", [128, NET, ROWS], BF16, kind="ExternalInput").ap()
    wkT = nc.dram_tensor("wkT", [128, NET, ROWS], BF16, kind="ExternalInput").ap()
    wvT = nc.dram_tensor("wvT", [128, NET, ROWS], BF16, kind="ExternalInput").ap()
    woT = nc.dram_tensor("woT", [ROWS, EMB], BF16, kind="ExternalInput").ap()
    maskT = nc.dram_tensor("maskT", [HPC, L, L], FP8, kind="ExternalInput").ap()
    outT = nc.dram_tensor("outT", [EMB, L], BF16, kind="ExternalOutput").ap()

    with tile.TileContext(nc) as tc:
        _mha_kernel(tc, outT, qT, kT, vT, wqT, wkT, wvT, woT, maskT)
    nc.compile()
    _CACHE["nc"] = nc
    return nc


def _pack_w(w):
    # [ROWS, EMB] -> w.T [EMB, ROWS] -> [128, NET, ROWS] with e = o*128+p
    return np.ascontiguousarray(
        w.T.reshape(NET, 128, ROWS).transpose(1, 0, 2)
    ).astype(NPBF16)


def _prep_in_maps(q, k, v, mask, Wq, Wk, Wv, Wo):
    qT = np.ascontiguousarray(q.T).astype(NPBF16)
    kT = np.ascontiguousarray(k.T).astype(NPBF16)
    vT = np.ascontiguousarray(v.T).astype(NPBF16)
    in_maps = []
    for c in range(NCORES):
        rows = slice(c * ROWS, (c + 1) * ROWS)
        in_maps.append({
            "qT": qT,
            "kT": kT,
            "vT": vT,
            "wqT": _pack_w(Wq[rows] * SCALE),
            "wkT": _pack_w(Wk[rows]),
            "wvT": _pack_w(Wv[rows]),
            "woT": np.ascontiguousarray(Wo[:, rows].T).astype(NPBF16),
            "maskT": np.ascontiguousarray(
                (~mask[c * HPC : (c + 1) * HPC]).swapaxes(1, 2)
            ).astype(NPFP8),
        })
    return in_maps


def run(q, k, v, mask, Wq, Wk, Wv, Wo, **spmd_kwargs):
    nc = _build()
    in_maps = _prep_in_maps(q, k, v, mask, Wq, Wk, Wv, Wo)
    res = run_bass_kernel_spmd(nc, in_maps, list(range(NCORES)), **spmd_kwargs)
    outT = np.zeros((EMB, L), np.float64)
    for r in res.results:
        outT += np.asarray(r["outT"]).astype(np.float64)
    out = np.ascontiguousarray(outT.T).astype(np.float32)
    return out, res


def kernel(q, k, v, mask, Wq, Wk, Wv, Wo):
    q, k, v = (np.asarray(x, np.float32) for x in (q, k, v))
    Wq, Wk, Wv, Wo = (np.asarray(x, np.float32) for x in (Wq, Wk, Wv, Wo))
    mask = np.asarray(mask, bool)
    out, _ = run(q, k, v, mask, Wq, Wk, Wv, Wo)
    return out
